# revision 1
# baseline (speedup 1.0000x reference)
"""DeltaOnlyModel Trainium2 kernel.

Pure data parallel over batch: 256 examples -> 8 cores x 32.
The per-token encoder collapses to 64-entry token tables (vocab=64, no
position mixing), computed on device. The gated delta-rule scan runs in
chunks of C=128 steps: per-chunk per-example K/theta sequences come from
one-hot matmuls on the tensor engine (quadrant packed), the sequential
gate recurrence runs on the vector engine in [32 ex x 64 H] layout with
fused scalar_tensor_tensor ops, and cross-sub-chunk corrections plus
fast-weight (M) updates are per-example matmuls accumulating in PSUM.
"""

import os
import numpy as np

H = 64
VOC = 64
L = 2048
B = 256
NCORE = 8
BPC = B // NCORE          # 32 examples per core
C = 128                   # chunk length (steps)
SC = 8                    # sub-chunk length (solve window)
NSUB = C // SC
THR2 = 0.4 * 0.4
LN_EPS = 1e-5
NORM_EPS = 1e-12

# test hook: truncate the scan to fewer chunks (kernel then models a
# shorter sequence whose readout token is x[:, NCH*C-1])
NCH = int(os.environ.get("KERNEL_NCH", L // C))
LEFF = NCH * C


def _build(nc, tc, ctx):
    from concourse import mybir
    f32 = mybir.dt.float32
    AL = mybir.AluOpType
    AF = mybir.ActivationFunctionType
    HB = BPC // 2   # examples per partition-half

    def inp(name, shape):
        return nc.dram_tensor(name, shape, f32, kind="ExternalInput").ap()

    xf = inp("xf", [BPC, L])
    embed = inp("embed", [VOC, H])
    w1 = inp("w1", [H, 2 * H])
    b1 = inp("b1", [2 * H, 1])
    w2 = inp("w2", [2 * H, H])
    b2 = inp("b2", [H, 1])
    ln_g = inp("ln_g", [1, H])
    ln_b = inp("ln_b", [1, H])
    wk = inp("wk", [H, H])
    wv = inp("wv", [H, H])
    wq = inp("wq", [H, H])
    wo = inp("wo", [H, H])
    bo = inp("bo", [H, 1])
    xl = inp("xl", [1, BPC])              # tokens at step LEFF-1
    iota2 = inp("iota2", [128, 1])        # p % 64
    ident = inp("ident", [128, 128])
    out_d = nc.dram_tensor("out", [H, BPC], f32, kind="ExternalOutput").ap()

    tabs_d = nc.dram_tensor("tabs_d", [VOC, 3 * H + 1], f32).ap()
    vt_d = nc.dram_tensor("vt_d", [VOC, H], f32).ap()
    aneg_d = nc.dram_tensor("aneg_d", [2, BPC, C, C], f32).ap()
    u_d = nc.dram_tensor("u_d", [2, C, BPC, H], f32).ap()
    r_d = nc.dram_tensor("r_d", [2, BPC, SC, H], f32).ap()

    cst = ctx.enter_context(tc.tile_pool(name="cst", bufs=1))
    tabp = ctx.enter_context(tc.tile_pool(name="tabp", bufs=1))
    big = ctx.enter_context(tc.tile_pool(name="big", bufs=1))
    dbl = ctx.enter_context(tc.tile_pool(name="dbl", bufs=2))
    sol = ctx.enter_context(tc.tile_pool(name="sol", bufs=3))
    smp = ctx.enter_context(tc.tile_pool(name="smp", bufs=3))
    stp = ctx.enter_context(tc.tile_pool(name="stp", bufs=2))
    rrp = ctx.enter_context(tc.tile_pool(name="rrp", bufs=2))
    pst = ctx.enter_context(tc.tile_pool(name="pst", bufs=4, space="PSUM"))
    psr = ctx.enter_context(tc.tile_pool(name="psr", bufs=1, space="PSUM"))

    # ============ phase 0: token tables ============
    embT = cst.tile([H, VOC], f32)
    nc.sync.dma_start(embT[:], embed.rearrange("a b -> b a"))
    w1s = cst.tile([H, 2 * H], f32)
    nc.sync.dma_start(w1s[:], w1)
    b1s = cst.tile([2 * H, 1], f32)
    nc.sync.dma_start(b1s[:], b1)
    w2s = cst.tile([2 * H, H], f32)
    nc.sync.dma_start(w2s[:], w2)
    b2s = cst.tile([H, 1], f32)
    nc.sync.dma_start(b2s[:], b2)
    gRow = cst.tile([VOC, H], f32)
    nc.sync.dma_start(gRow[:], ln_g.broadcast_to([VOC, H]))
    bRow = cst.tile([VOC, H], f32)
    nc.sync.dma_start(bRow[:], ln_b.broadcast_to([VOC, H]))
    wks = cst.tile([H, H], f32)
    nc.sync.dma_start(wks[:], wk)
    wvs = cst.tile([H, H], f32)
    nc.sync.dma_start(wvs[:], wv)
    wqs = cst.tile([H, H], f32)
    nc.sync.dma_start(wqs[:], wq)
    wos = cst.tile([H, H], f32)
    nc.sync.dma_start(wos[:], wo)
    bos = cst.tile([H, 1], f32)
    nc.sync.dma_start(bos[:], bo)
    iot = cst.tile([128, 1], f32)
    nc.sync.dma_start(iot[:], iota2)
    idn = cst.tile([128, 128], f32)
    nc.sync.dma_start(idn[:], ident)

    ps1 = pst.tile([2 * H, VOC], f32, tag="ps")
    nc.tensor.matmul(ps1[:], w1s[:], embT[:], start=True, stop=True)
    r1 = smp.tile([2 * H, VOC], f32)
    nc.scalar.activation(r1[:], ps1[:], AF.Relu, bias=b1s[:], scale=1.0)
    ps2 = pst.tile([H, VOC], f32, tag="ps")
    nc.tensor.matmul(ps2[:], w2s[:], r1[:], start=True, stop=True)
    hpreT = smp.tile([H, VOC], f32)
    nc.vector.scalar_tensor_tensor(hpreT[:], ps2[:], b2s[:], embT[:],
                                   op0=AL.add, op1=AL.add)
    ps3 = pst.tile([VOC, H], f32, tag="ps")
    nc.tensor.transpose(ps3[:], hpreT[:], idn[0:H, 0:VOC])
    hp = smp.tile([VOC, H], f32)
    nc.scalar.copy(hp[:], ps3[:])
    mu = smp.tile([VOC, 1], f32)
    nc.vector.reduce_sum(mu[:], hp[:], axis=mybir.AxisListType.X)
    nc.vector.tensor_scalar(mu[:], mu[:], 1.0 / H, None, op0=AL.mult)
    xc = smp.tile([VOC, H], f32)
    nc.vector.tensor_scalar(xc[:], hp[:], mu[:], None, op0=AL.subtract)
    var = smp.tile([VOC, 1], f32)
    sq = smp.tile([VOC, H], f32)
    nc.vector.scalar_tensor_tensor(sq[:], xc[:], 1.0, xc[:],
                                   op0=AL.bypass, op1=AL.mult, accum_out=var[:])
    rstd = smp.tile([VOC, 1], f32)
    nc.vector.tensor_scalar(rstd[:], var[:], 1.0 / H, LN_EPS,
                            op0=AL.mult, op1=AL.add)
    nc.scalar.activation(rstd[:], rstd[:], AF.Sqrt)
    nc.vector.reciprocal(rstd[:], rstd[:])
    hn = smp.tile([VOC, H], f32)
    nc.vector.tensor_scalar(hn[:], xc[:], rstd[:], None, op0=AL.mult)
    nc.vector.tensor_mul(hn[:], hn[:], gRow[:])
    nc.vector.tensor_add(hn[:], hn[:], bRow[:])
    ps4 = pst.tile([H, VOC], f32, tag="ps")
    nc.tensor.transpose(ps4[:], hn[:], idn[0:VOC, 0:H])
    hnT = smp.tile([H, VOC], f32)
    nc.scalar.copy(hnT[:], ps4[:])

    psk = pst.tile([VOC, 3 * H], f32, tag="ps")
    nc.tensor.matmul(psk[:, 0:H], hnT[:], wks[:], start=True, stop=True)
    nc.tensor.matmul(psk[:, H:2 * H], hnT[:], wvs[:], start=True, stop=True)
    nc.tensor.matmul(psk[:, 2 * H:3 * H], hnT[:], wqs[:], start=True, stop=True)
    kvq = smp.tile([VOC, 3 * H], f32)
    nc.vector.tensor_copy(kvq[:], psk[:])
    kn2 = smp.tile([VOC, 1], f32)
    ksq = smp.tile([VOC, H], f32)
    nc.vector.scalar_tensor_tensor(ksq[:], kvq[:, 0:H], 1.0, kvq[:, 0:H],
                                   op0=AL.bypass, op1=AL.mult, accum_out=kn2[:])
    rkn = smp.tile([VOC, 1], f32)
    nc.scalar.activation(rkn[:], kn2[:], AF.Sqrt)
    nc.vector.tensor_scalar(rkn[:], rkn[:], NORM_EPS, None, op0=AL.max)
    nc.vector.reciprocal(rkn[:], rkn[:])
    rknn = smp.tile([VOC, 1], f32)
    nc.vector.tensor_scalar(rknn[:], rkn[:], -1.0, None, op0=AL.mult)

    tabs = smp.tile([VOC, 3 * H + 1], f32)
    nc.vector.tensor_scalar(tabs[:, 0:H], kvq[:, 0:H], rkn[:], None, op0=AL.mult)
    nc.vector.tensor_scalar(tabs[:, H:2 * H], kvq[:, 0:H], rknn[:], None,
                            op0=AL.mult)
    nc.vector.tensor_scalar(tabs[:, 2 * H:3 * H], kvq[:, 2 * H:3 * H], -1.0,
                            None, op0=AL.mult)
    vtab = smp.tile([VOC, H], f32)
    nc.vector.tensor_copy(vtab[:], kvq[:, H:2 * H])
    th1 = smp.tile([VOC, 1], f32)
    vsq = smp.tile([VOC, H], f32)
    nc.vector.scalar_tensor_tensor(vsq[:], vtab[:], 1.0, vtab[:],
                                   op0=AL.bypass, op1=AL.mult, accum_out=th1[:])
    nc.vector.tensor_scalar(tabs[:, 3 * H:3 * H + 1], th1[:], THR2, None,
                            op0=AL.mult)

    nc.sync.dma_start(tabs_d, tabs[:])
    nc.sync.dma_start(vt_d, vtab[:])
    tab2 = tabp.tile([128, 3 * H + 1], f32)
    nc.sync.dma_start(tab2[0:VOC, :], tabs_d)
    nc.sync.dma_start(tab2[VOC:128, :], tabs_d)
    vtab2 = tabp.tile([128, H], f32)
    nc.sync.dma_start(vtab2[0:VOC, :], vt_d)
    nc.sync.dma_start(vtab2[VOC:128, :], vt_d)

    Mneg = tabp.tile([H, BPC * H], f32)   # -M^T per example
    nc.vector.memzero(Mneg[:])

    # ============ phase 1: chunks ============
    for ci in range(NCH):
        db = ci % 2
        cs = ci * C
        xb = big.tile([128, HB, C], f32, tag="xb")
        nc.sync.dma_start(xb[0:VOC, :, :],
                          xf[0:HB, cs:cs + C].unsqueeze(0)
                          .broadcast_to([VOC, HB, C]))
        nc.sync.dma_start(xb[VOC:128, :, :],
                          xf[HB:BPC, cs:cs + C].unsqueeze(0)
                          .broadcast_to([VOC, HB, C]))
        oh = big.tile([128, HB, C], f32, tag="oh")
        nc.vector.tensor_scalar(oh[:], xb[:], iot[:], None, op0=AL.is_equal)

        kall = dbl.tile([C, BPC, 2 * H], f32, tag="kall")
        thcol_all = smp.tile([C, BPC], f32, tag="thcol")
        ktall = dbl.tile([H, BPC, C], f32, tag="ktall")
        ktnall = big.tile([H, BPC, C], f32, tag="ktnall")
        for g in range(BPC // 4):
            psa = pst.tile([C, 4, 2 * H], f32, tag="ps")
            psth = pst.tile([C, 4, 1], f32, tag="ps")
            psbT = pst.tile([H, 4, C], f32, tag="ps")
            psbTn = pst.tile([H, 4, C], f32, tag="ps")
            for j in range(4):
                e = g * 4 + j
                half = 0 if e < HB else VOC
                es = e if e < HB else e - HB
                ohs = oh[half:half + VOC, es, :]
                nc.tensor.matmul(psa[:, j, :], ohs,
                                 tab2[half:half + VOC, 0:2 * H],
                                 start=True, stop=True, tile_position=(half, 0))
                nc.tensor.matmul(psth[:, j, :], ohs,
                                 tab2[half:half + VOC, 3 * H:3 * H + 1],
                                 start=True, stop=True, tile_position=(half, 0))
                nc.tensor.matmul(psbT[:, j, :],
                                 tab2[half:half + VOC, 0:H], ohs,
                                 start=True, stop=True, tile_position=(half, 0))
                nc.tensor.matmul(psbTn[:, j, :],
                                 tab2[half:half + VOC, H:2 * H], ohs,
                                 start=True, stop=True, tile_position=(half, 0))
            nc.scalar.copy(kall[:, g * 4:(g + 1) * 4, :], psa[:])
            nc.scalar.copy(thcol_all[:, g * 4:(g + 1) * 4], psth[:, :, 0])
            nc.scalar.copy(ktall[:, g * 4:(g + 1) * 4, :], psbT[:])
            nc.scalar.copy(ktnall[:, g * 4:(g + 1) * 4, :], psbTn[:])

        thps = pst.tile([BPC, C], f32, tag="ps")
        nc.tensor.transpose(thps[:], thcol_all[:], idn[0:C, 0:C])
        thb = sol.tile([BPC, C], f32, tag="thb")
        nc.scalar.copy(thb[:], thps[:])

        for g in range(BPC // 4):
            pan = pst.tile([C, 4, C], f32, tag="ps")
            ansb = smp.tile([C, 4, C], f32, tag="ansb")
            for j in range(4):
                e = g * 4 + j
                nc.tensor.matmul(pan[:, j, :], ktall[:, e, :], ktnall[:, e, :],
                                 start=True, stop=True)
            nc.scalar.copy(ansb[:], pan[:])
            nc.sync.dma_start(aneg_d[db, g * 4:(g + 1) * 4].transpose([1, 0, 2]),
                              ansb[:])
        acols = sol.tile([BPC, NSUB * SC, SC], f32, tag="acols")
        for J in range(NSUB):
            nc.sync.dma_start(
                acols[:, J * SC:(J + 1) * SC, :],
                aneg_d[db, :, J * SC:(J + 1) * SC, J * SC:(J + 1) * SC])

        # r'' init: V gather, then -K M^T
        # PSUM group discipline: one start=True per bank (8 examples/bank)
        # per chunk; everything else accumulates via per-element has_written.
        rps = psr.tile([C, BPC, H], f32, tag="rps")
        for e in range(BPC):
            half = 0 if e < HB else VOC
            es = e if e < HB else e - HB
            ohs = oh[half:half + VOC, es, :]
            nc.tensor.matmul(rps[:, e, :], ohs, vtab2[half:half + VOC, :],
                             start=(e % 8 == 0), stop=False,
                             tile_position=(half, 0), skip_group_check=True)
        laststop = NSUB < 2
        for e in range(BPC):
            nc.tensor.matmul(rps[:, e, :], ktall[:, e, :],
                             Mneg[:, e * H:(e + 1) * H],
                             start=False, stop=(laststop and e % 8 == 7),
                             skip_group_check=True)

        uc = big.tile([C, BPC, H], f32, tag="uc")
        for J in range(NSUB):
            rrow = rrp.tile([C, BPC, H], f32, tag="rrow")
            nc.scalar.copy(rrow[:], rps[:])
            nc.sync.dma_start(r_d[db].transpose([1, 0, 2]),
                              rrow[J * SC:(J + 1) * SC, :, :])
            rb = sol.tile([BPC, SC, H], f32, tag="rb")
            nc.sync.dma_start(rb[:, 0:1, :], r_d[db, :, 0:1, :])
            nc.sync.dma_start(rb[:, 1:SC, :], r_d[db, :, 1:SC, :])

            for k in range(SC):
                t = J * SC + k
                dslot = rb[:, k, :]
                for s in range(k):
                    nc.vector.scalar_tensor_tensor(
                        dslot, rb[:, s, :], acols[:, J * SC + s, k:k + 1], dslot,
                        op0=AL.mult, op1=AL.add)
                nsc = smp.tile([BPC, H], f32, tag="nsc")
                ncol = smp.tile([BPC, 1], f32, tag="ncol")
                nc.vector.scalar_tensor_tensor(nsc[:], dslot, 1.0, dslot,
                                               op0=AL.bypass, op1=AL.mult,
                                               accum_out=ncol[:])
                nc.vector.scalar_tensor_tensor(
                    dslot, ncol[:].broadcast_to([BPC, H]), thb[:, t:t + 1],
                    dslot, op0=AL.is_gt, op1=AL.mult)
                if k == SC - 2:
                    nc.sync.dma_start(
                        u_d[db, J * SC:J * SC + SC - 1].transpose([1, 0, 2]),
                        rb[:, 0:SC - 1, :])
            nc.sync.dma_start(
                u_d[db, J * SC + SC - 1:(J + 1) * SC].transpose([1, 0, 2]),
                rb[:, SC - 1:SC, :])
            if J + 1 < NSUB:
                # 4-way packed strip corrections: example groups at
                # partition rows 0/32/64/96, future-rows-only stationary.
                J32 = ((J + 1) * SC // 64) * 64
                usub = stp.tile([128, BPC // 4, H], f32, tag="usub")
                strip = stp.tile([128, BPC // 4, C], f32, tag="strip")
                for q in range(4):
                    es, ee = q * 8, (q + 1) * 8
                    nc.sync.dma_start(usub[32 * q:32 * q + SC, :, :],
                                      u_d[db, J * SC:(J + 1) * SC, es:ee, :]
                                      .transpose([0, 1, 2]))
                    nc.sync.dma_start(
                        strip[32 * q:32 * q + SC, :, :],
                        aneg_d[db, es:ee, J * SC:(J + 1) * SC, :]
                        .transpose([1, 0, 2]))
                for e in range(BPC):
                    q, er = e // 8, e % 8
                    nc.tensor.matmul(rps[J32:C, e, :],
                                     strip[32 * q:32 * q + SC, er, J32:C],
                                     usub[32 * q:32 * q + SC, er, :],
                                     start=False,
                                     stop=(J == NSUB - 2 and e % 8 == 7),
                                     skip_group_check=True,
                                     tile_position=(32 * q, J32))

        nc.sync.dma_start(uc[:], u_d[db])
        dmp = psr.tile([H, BPC, H], f32, tag="rps")
        for e in range(BPC):
            nc.tensor.matmul(dmp[:, e, :], kall[:, e, H:2 * H], uc[:, e, :],
                             start=True, stop=True)
        nc.vector.tensor_add(Mneg[:], Mneg[:],
                             dmp[:].rearrange("j e h -> j (e h)"))

    # ============ phase 2: readout ============
    xlb = smp.tile([VOC, BPC], f32, tag="xlb")
    nc.sync.dma_start(xlb[:], xl.broadcast_to([VOC, BPC]))
    ohl = smp.tile([VOC, BPC], f32, tag="ohl")
    nc.vector.tensor_scalar(ohl[:], xlb[:], iot[0:VOC, :], None,
                            op0=AL.is_equal)
    psq = pst.tile([H, BPC], f32, tag="ps")
    nc.tensor.matmul(psq[:], tab2[0:VOC, 2 * H:3 * H], ohl[:],
                     start=True, stop=True)
    qng = smp.tile([H, BPC], f32, tag="qng")
    nc.scalar.copy(qng[:], psq[:])
    prd = pst.tile([H, BPC], f32, tag="ps")
    for e in range(BPC):
        nc.tensor.matmul(prd[:, e:e + 1], Mneg[:, e * H:(e + 1) * H],
                         qng[:, e:e + 1], start=True, stop=True)
    rd = smp.tile([H, BPC], f32, tag="rd")
    nc.scalar.activation(rd[:], prd[:], AF.Relu)
    pso = pst.tile([H, BPC], f32, tag="ps")
    nc.tensor.matmul(pso[:], wos[:], rd[:], start=True, stop=True)
    ot = smp.tile([H, BPC], f32, tag="ot")
    nc.vector.tensor_scalar(ot[:], pso[:], bos[:], None, op0=AL.add)
    nc.sync.dma_start(out_d, ot[:])


def build_nc():
    from concourse import bacc
    import concourse.tile as tile
    from contextlib import ExitStack
    nc = bacc.Bacc("TRN2", target_bir_lowering=False, debug=False,
                   num_devices=NCORE)
    with tile.TileContext(nc) as tc:
        with ExitStack() as ctx:
            _build(nc, tc, ctx)
    nc.compile()
    return nc


def make_in_maps(inputs):
    x = np.asarray(inputs["x"]).astype(np.int64)
    consts = {
        "embed": inputs["embed"], "w1": inputs["w1"],
        "b1": np.asarray(inputs["b1"]).reshape(2 * H, 1),
        "w2": inputs["w2"], "b2": np.asarray(inputs["b2"]).reshape(H, 1),
        "ln_g": np.asarray(inputs["ln_g"]).reshape(1, H),
        "ln_b": np.asarray(inputs["ln_b"]).reshape(1, H),
        "wk": inputs["wk"], "wv": inputs["wv"], "wq": inputs["wq"],
        "wo": inputs["wo"], "bo": np.asarray(inputs["bo"]).reshape(H, 1),
        "iota2": (np.arange(128) % 64).astype(np.float32).reshape(128, 1),
        "ident": np.eye(128, dtype=np.float32),
    }
    consts = {k: np.ascontiguousarray(np.asarray(v, dtype=np.float32))
              for k, v in consts.items()}
    in_maps = []
    for c in range(NCORE):
        m = dict(consts)
        m["xf"] = np.ascontiguousarray(
            x[c * BPC:(c + 1) * BPC].astype(np.float32))
        m["xl"] = np.ascontiguousarray(
            x[c * BPC:(c + 1) * BPC, LEFF - 1].astype(np.float32).reshape(1, BPC))
        in_maps.append(m)
    return in_maps


def kernel(**inputs):
    from concourse.bass_utils import run_bass_kernel_spmd
    nc = build_nc()
    in_maps = make_in_maps(inputs)
    res = run_bass_kernel_spmd(nc, in_maps, list(range(NCORE)))
    outs = []
    for c in range(NCORE):
        o = np.asarray(res.results[c]["out"])   # [H, BPC]
        outs.append(o.T)
    return np.concatenate(outs, axis=0).astype(np.float32)



# revision 3
# speedup vs baseline: 4.1087x; 4.1087x over previous
"""DeltaOnlyModel Trainium2 kernel.

Pure data parallel over batch: 256 examples -> 8 cores x 32.
The per-token encoder collapses to 64-entry token tables (vocab=64, no
position mixing), computed on device. The gated delta-rule scan runs in
chunks of C=128 steps: per-chunk per-example K/theta sequences come from
one-hot matmuls on the tensor engine (quadrant packed), the sequential
gate recurrence runs on the vector engine in [32 ex x 64 H] layout with
fused scalar_tensor_tensor ops, and cross-sub-chunk corrections plus
fast-weight (M) updates are per-example matmuls accumulating in PSUM.
"""

import os
import numpy as np

H = 64
VOC = 64
L = 2048
B = 256
NCORE = 8
BPC = B // NCORE          # 32 examples per core
C = 128                   # chunk length (steps)
SC = 8                    # sub-chunk length (solve window)
NSUB = C // SC
THR2 = 0.4 * 0.4
LN_EPS = 1e-5
NORM_EPS = 1e-12

# test hook: truncate the scan to fewer chunks (kernel then models a
# shorter sequence whose readout token is x[:, NCH*C-1])
NCH = int(os.environ.get("KERNEL_NCH", L // C))
LEFF = NCH * C


def _build(nc, tc, ctx):
    from concourse import mybir
    f32 = mybir.dt.float32
    AL = mybir.AluOpType
    AF = mybir.ActivationFunctionType
    HB = BPC // 2   # examples per partition-half

    def inp(name, shape):
        return nc.dram_tensor(name, shape, f32, kind="ExternalInput").ap()

    xf = inp("xf", [BPC, L])
    embed = inp("embed", [VOC, H])
    w1 = inp("w1", [H, 2 * H])
    b1 = inp("b1", [2 * H, 1])
    w2 = inp("w2", [2 * H, H])
    b2 = inp("b2", [H, 1])
    ln_g = inp("ln_g", [1, H])
    ln_b = inp("ln_b", [1, H])
    wk = inp("wk", [H, H])
    wv = inp("wv", [H, H])
    wq = inp("wq", [H, H])
    wo = inp("wo", [H, H])
    bo = inp("bo", [H, 1])
    xl = inp("xl", [1, BPC])              # tokens at step LEFF-1
    iota2 = inp("iota2", [128, 1])        # p % 64
    ident = inp("ident", [128, 128])
    out_d = nc.dram_tensor("out", [H, BPC], f32, kind="ExternalOutput").ap()

    tabs_d = nc.dram_tensor("tabs_d", [VOC, 3 * H + 1], f32).ap()
    vt_d = nc.dram_tensor("vt_d", [VOC, H], f32).ap()
    aneg_d = nc.dram_tensor("aneg_d", [2, BPC, C, C], f32).ap()
    u_d = nc.dram_tensor("u_d", [2, C, BPC, H], f32).ap()
    r_d = nc.dram_tensor("r_d", [2, BPC, SC, H], f32).ap()

    cst = ctx.enter_context(tc.tile_pool(name="cst", bufs=1))
    tabp = ctx.enter_context(tc.tile_pool(name="tabp", bufs=1))
    big = ctx.enter_context(tc.tile_pool(name="big", bufs=1))
    dbl = ctx.enter_context(tc.tile_pool(name="dbl", bufs=2))
    sol = ctx.enter_context(tc.tile_pool(name="sol", bufs=3))
    smp = ctx.enter_context(tc.tile_pool(name="smp", bufs=3))
    stp = ctx.enter_context(tc.tile_pool(name="stp", bufs=2))
    rrp = ctx.enter_context(tc.tile_pool(name="rrp", bufs=2))
    pst = ctx.enter_context(tc.tile_pool(name="pst", bufs=4, space="PSUM"))
    psr = ctx.enter_context(tc.tile_pool(name="psr", bufs=1, space="PSUM"))

    # ============ phase 0: token tables ============
    embT = cst.tile([H, VOC], f32)
    nc.sync.dma_start(embT[:], embed.rearrange("a b -> b a"))
    w1s = cst.tile([H, 2 * H], f32)
    nc.sync.dma_start(w1s[:], w1)
    b1s = cst.tile([2 * H, 1], f32)
    nc.sync.dma_start(b1s[:], b1)
    w2s = cst.tile([2 * H, H], f32)
    nc.sync.dma_start(w2s[:], w2)
    b2s = cst.tile([H, 1], f32)
    nc.sync.dma_start(b2s[:], b2)
    gRow = cst.tile([VOC, H], f32)
    nc.sync.dma_start(gRow[:], ln_g.broadcast_to([VOC, H]))
    bRow = cst.tile([VOC, H], f32)
    nc.sync.dma_start(bRow[:], ln_b.broadcast_to([VOC, H]))
    wks = cst.tile([H, H], f32)
    nc.sync.dma_start(wks[:], wk)
    wvs = cst.tile([H, H], f32)
    nc.sync.dma_start(wvs[:], wv)
    wqs = cst.tile([H, H], f32)
    nc.sync.dma_start(wqs[:], wq)
    wos = cst.tile([H, H], f32)
    nc.sync.dma_start(wos[:], wo)
    bos = cst.tile([H, 1], f32)
    nc.sync.dma_start(bos[:], bo)
    iot = cst.tile([128, 1], f32)
    nc.sync.dma_start(iot[:], iota2)
    idn = cst.tile([128, 128], f32)
    nc.sync.dma_start(idn[:], ident)

    ps1 = pst.tile([2 * H, VOC], f32, tag="ps")
    nc.tensor.matmul(ps1[:], w1s[:], embT[:], start=True, stop=True)
    r1 = smp.tile([2 * H, VOC], f32)
    nc.scalar.activation(r1[:], ps1[:], AF.Relu, bias=b1s[:], scale=1.0)
    ps2 = pst.tile([H, VOC], f32, tag="ps")
    nc.tensor.matmul(ps2[:], w2s[:], r1[:], start=True, stop=True)
    hpreT = smp.tile([H, VOC], f32)
    nc.vector.scalar_tensor_tensor(hpreT[:], ps2[:], b2s[:], embT[:],
                                   op0=AL.add, op1=AL.add)
    ps3 = pst.tile([VOC, H], f32, tag="ps")
    nc.tensor.transpose(ps3[:], hpreT[:], idn[0:H, 0:VOC])
    hp = smp.tile([VOC, H], f32)
    nc.scalar.copy(hp[:], ps3[:])
    mu = smp.tile([VOC, 1], f32)
    nc.vector.reduce_sum(mu[:], hp[:], axis=mybir.AxisListType.X)
    nc.vector.tensor_scalar(mu[:], mu[:], 1.0 / H, None, op0=AL.mult)
    xc = smp.tile([VOC, H], f32)
    nc.vector.tensor_scalar(xc[:], hp[:], mu[:], None, op0=AL.subtract)
    var = smp.tile([VOC, 1], f32)
    sq = smp.tile([VOC, H], f32)
    nc.vector.scalar_tensor_tensor(sq[:], xc[:], 1.0, xc[:],
                                   op0=AL.bypass, op1=AL.mult, accum_out=var[:])
    rstd = smp.tile([VOC, 1], f32)
    nc.vector.tensor_scalar(rstd[:], var[:], 1.0 / H, LN_EPS,
                            op0=AL.mult, op1=AL.add)
    nc.scalar.activation(rstd[:], rstd[:], AF.Sqrt)
    nc.vector.reciprocal(rstd[:], rstd[:])
    hn = smp.tile([VOC, H], f32)
    nc.vector.tensor_scalar(hn[:], xc[:], rstd[:], None, op0=AL.mult)
    nc.vector.tensor_mul(hn[:], hn[:], gRow[:])
    nc.vector.tensor_add(hn[:], hn[:], bRow[:])
    ps4 = pst.tile([H, VOC], f32, tag="ps")
    nc.tensor.transpose(ps4[:], hn[:], idn[0:VOC, 0:H])
    hnT = smp.tile([H, VOC], f32)
    nc.scalar.copy(hnT[:], ps4[:])

    psk = pst.tile([VOC, 3 * H], f32, tag="ps")
    nc.tensor.matmul(psk[:, 0:H], hnT[:], wks[:], start=True, stop=True)
    nc.tensor.matmul(psk[:, H:2 * H], hnT[:], wvs[:], start=True, stop=True)
    nc.tensor.matmul(psk[:, 2 * H:3 * H], hnT[:], wqs[:], start=True, stop=True)
    kvq = smp.tile([VOC, 3 * H], f32)
    nc.vector.tensor_copy(kvq[:], psk[:])
    kn2 = smp.tile([VOC, 1], f32)
    ksq = smp.tile([VOC, H], f32)
    nc.vector.scalar_tensor_tensor(ksq[:], kvq[:, 0:H], 1.0, kvq[:, 0:H],
                                   op0=AL.bypass, op1=AL.mult, accum_out=kn2[:])
    rkn = smp.tile([VOC, 1], f32)
    nc.scalar.activation(rkn[:], kn2[:], AF.Sqrt)
    nc.vector.tensor_scalar(rkn[:], rkn[:], NORM_EPS, None, op0=AL.max)
    nc.vector.reciprocal(rkn[:], rkn[:])
    rknn = smp.tile([VOC, 1], f32)
    nc.vector.tensor_scalar(rknn[:], rkn[:], -1.0, None, op0=AL.mult)

    tabs = smp.tile([VOC, 3 * H + 1], f32)
    nc.vector.tensor_scalar(tabs[:, 0:H], kvq[:, 0:H], rkn[:], None, op0=AL.mult)
    nc.vector.tensor_scalar(tabs[:, H:2 * H], kvq[:, 0:H], rknn[:], None,
                            op0=AL.mult)
    nc.vector.tensor_scalar(tabs[:, 2 * H:3 * H], kvq[:, 2 * H:3 * H], -1.0,
                            None, op0=AL.mult)
    vtab = smp.tile([VOC, H], f32)
    nc.vector.tensor_copy(vtab[:], kvq[:, H:2 * H])
    th1 = smp.tile([VOC, 1], f32)
    vsq = smp.tile([VOC, H], f32)
    nc.vector.scalar_tensor_tensor(vsq[:], vtab[:], 1.0, vtab[:],
                                   op0=AL.bypass, op1=AL.mult, accum_out=th1[:])
    nc.vector.tensor_scalar(tabs[:, 3 * H:3 * H + 1], th1[:], THR2, None,
                            op0=AL.mult)

    nc.sync.dma_start(tabs_d, tabs[:])
    nc.sync.dma_start(vt_d, vtab[:])
    tab2 = tabp.tile([128, 3 * H + 1], f32)
    nc.sync.dma_start(tab2[0:VOC, :], tabs_d)
    nc.sync.dma_start(tab2[VOC:128, :], tabs_d)
    vtab2 = tabp.tile([128, H], f32)
    nc.sync.dma_start(vtab2[0:VOC, :], vt_d)
    nc.sync.dma_start(vtab2[VOC:128, :], vt_d)

    Mneg = tabp.tile([H, BPC * H], f32)   # -M^T per example
    nc.vector.memzero(Mneg[:])

    # ============ phase 1: chunks ============
    for ci in range(NCH):
        db = ci % 2
        cs = ci * C
        xb = big.tile([128, HB, C], f32, tag="xb")
        nc.sync.dma_start(xb[0:VOC, :, :],
                          xf[0:HB, cs:cs + C].unsqueeze(0)
                          .broadcast_to([VOC, HB, C]))
        nc.sync.dma_start(xb[VOC:128, :, :],
                          xf[HB:BPC, cs:cs + C].unsqueeze(0)
                          .broadcast_to([VOC, HB, C]))
        oh = big.tile([128, HB, C], f32, tag="oh")
        nc.vector.tensor_scalar(oh[:], xb[:], iot[:], None, op0=AL.is_equal)

        kall = dbl.tile([C, BPC, 2 * H], f32, tag="kall")
        thcol_all = smp.tile([C, BPC], f32, tag="thcol")
        ktall = dbl.tile([H, BPC, C], f32, tag="ktall")
        ktnall = big.tile([H, BPC, C], f32, tag="ktnall")
        for g in range(BPC // 4):
            psa = pst.tile([C, 4, 2 * H], f32, tag="ps")
            psth = pst.tile([C, 4, 1], f32, tag="ps")
            psbT = pst.tile([H, 4, C], f32, tag="ps")
            psbTn = pst.tile([H, 4, C], f32, tag="ps")
            for j in range(4):
                e = g * 4 + j
                half = 0 if e < HB else VOC
                es = e if e < HB else e - HB
                ohs = oh[half:half + VOC, es, :]
                nc.tensor.matmul(psa[:, j, :], ohs,
                                 tab2[half:half + VOC, 0:2 * H],
                                 start=True, stop=True, tile_position=(half, 0))
                nc.tensor.matmul(psth[:, j, :], ohs,
                                 tab2[half:half + VOC, 3 * H:3 * H + 1],
                                 start=True, stop=True, tile_position=(half, 0))
                nc.tensor.matmul(psbT[:, j, :],
                                 tab2[half:half + VOC, 0:H], ohs,
                                 start=True, stop=True, tile_position=(half, 0))
                nc.tensor.matmul(psbTn[:, j, :],
                                 tab2[half:half + VOC, H:2 * H], ohs,
                                 start=True, stop=True, tile_position=(half, 0))
            nc.scalar.copy(kall[:, g * 4:(g + 1) * 4, :], psa[:])
            nc.scalar.copy(thcol_all[:, g * 4:(g + 1) * 4], psth[:, :, 0])
            nc.scalar.copy(ktall[:, g * 4:(g + 1) * 4, :], psbT[:])
            nc.scalar.copy(ktnall[:, g * 4:(g + 1) * 4, :], psbTn[:])

        thps = pst.tile([BPC, C], f32, tag="ps")
        nc.tensor.transpose(thps[:], thcol_all[:], idn[0:C, 0:C])
        thb = sol.tile([BPC, C], f32, tag="thb")
        nc.scalar.copy(thb[:], thps[:])

        for g in range(BPC // 4):
            pan = pst.tile([C, 4, C], f32, tag="ps")
            ansb = smp.tile([C, 4, C], f32, tag="ansb")
            for j in range(4):
                e = g * 4 + j
                nc.tensor.matmul(pan[:, j, :], ktall[:, e, :], ktnall[:, e, :],
                                 start=True, stop=True)
            nc.scalar.copy(ansb[:], pan[:])
            nc.sync.dma_start(aneg_d[db, g * 4:(g + 1) * 4].transpose([1, 0, 2]),
                              ansb[:])
        acols = sol.tile([BPC, NSUB * SC, SC], f32, tag="acols")
        for J in range(NSUB):
            nc.sync.dma_start(
                acols[:, J * SC:(J + 1) * SC, :],
                aneg_d[db, :, J * SC:(J + 1) * SC, J * SC:(J + 1) * SC])

        # r'' init: V gather, then -K M^T
        # PSUM group discipline: one start=True per bank (8 examples/bank)
        # per chunk; everything else accumulates via per-element has_written.
        rps = psr.tile([C, BPC, H], f32, tag="rps")
        for e in range(BPC):
            half = 0 if e < HB else VOC
            es = e if e < HB else e - HB
            ohs = oh[half:half + VOC, es, :]
            nc.tensor.matmul(rps[:, e, :], ohs, vtab2[half:half + VOC, :],
                             start=(e % 8 == 0), stop=False,
                             tile_position=(half, 0), skip_group_check=True)
        laststop = NSUB < 2
        for e in range(BPC):
            nc.tensor.matmul(rps[:, e, :], ktall[:, e, :],
                             Mneg[:, e * H:(e + 1) * H],
                             start=False, stop=(laststop and e % 8 == 7),
                             skip_group_check=True)

        uc = big.tile([C, BPC, H], f32, tag="uc")
        for J in range(NSUB):
            rrow = rrp.tile([C, BPC, H], f32, tag="rrow")
            nc.scalar.copy(rrow[:], rps[:])
            nc.sync.dma_start(r_d[db].transpose([1, 0, 2]),
                              rrow[J * SC:(J + 1) * SC, :, :])
            rb = sol.tile([BPC, SC, H], f32, tag="rb")
            nc.sync.dma_start(rb[:, 0:1, :], r_d[db, :, 0:1, :])
            nc.sync.dma_start(rb[:, 1:SC, :], r_d[db, :, 1:SC, :])

            for k in range(SC):
                t = J * SC + k
                dslot = rb[:, k, :]
                for s in range(k):
                    nc.vector.scalar_tensor_tensor(
                        dslot, rb[:, s, :], acols[:, J * SC + s, k:k + 1], dslot,
                        op0=AL.mult, op1=AL.add)
                nsc = smp.tile([BPC, H], f32, tag="nsc")
                ncol = smp.tile([BPC, 1], f32, tag="ncol")
                nc.vector.scalar_tensor_tensor(nsc[:], dslot, 1.0, dslot,
                                               op0=AL.bypass, op1=AL.mult,
                                               accum_out=ncol[:])
                nc.vector.scalar_tensor_tensor(
                    dslot, ncol[:].broadcast_to([BPC, H]), thb[:, t:t + 1],
                    dslot, op0=AL.is_gt, op1=AL.mult)
                if k == SC - 2:
                    nc.sync.dma_start(
                        u_d[db, J * SC:J * SC + SC - 1].transpose([1, 0, 2]),
                        rb[:, 0:SC - 1, :])
            nc.sync.dma_start(
                u_d[db, J * SC + SC - 1:(J + 1) * SC].transpose([1, 0, 2]),
                rb[:, SC - 1:SC, :])
            if J + 1 < NSUB:
                # 4-way packed strip corrections: example groups at
                # partition rows 0/32/64/96, future-rows-only stationary.
                J32 = ((J + 1) * SC // 64) * 64
                usub = stp.tile([128, BPC // 4, H], f32, tag="usub")
                strip = stp.tile([128, BPC // 4, C], f32, tag="strip")
                for q in range(4):
                    es, ee = q * 8, (q + 1) * 8
                    nc.sync.dma_start(usub[32 * q:32 * q + SC, :, :],
                                      u_d[db, J * SC:(J + 1) * SC, es:ee, :]
                                      .transpose([0, 1, 2]))
                    nc.sync.dma_start(
                        strip[32 * q:32 * q + SC, :, :],
                        aneg_d[db, es:ee, J * SC:(J + 1) * SC, :]
                        .transpose([1, 0, 2]))
                for e in range(BPC):
                    q, er = e // 8, e % 8
                    nc.tensor.matmul(rps[J32:C, e, :],
                                     strip[32 * q:32 * q + SC, er, J32:C],
                                     usub[32 * q:32 * q + SC, er, :],
                                     start=False,
                                     stop=(J == NSUB - 2 and e % 8 == 7),
                                     skip_group_check=True,
                                     tile_position=(32 * q, J32))

        nc.sync.dma_start(uc[:], u_d[db])
        dmp = psr.tile([H, BPC, H], f32, tag="rps")
        for e in range(BPC):
            nc.tensor.matmul(dmp[:, e, :], kall[:, e, H:2 * H], uc[:, e, :],
                             start=True, stop=True)
        nc.vector.tensor_add(Mneg[:], Mneg[:],
                             dmp[:].rearrange("j e h -> j (e h)"))

    # ============ phase 2: readout ============
    xlb = smp.tile([VOC, BPC], f32, tag="xlb")
    nc.sync.dma_start(xlb[:], xl.broadcast_to([VOC, BPC]))
    ohl = smp.tile([VOC, BPC], f32, tag="ohl")
    nc.vector.tensor_scalar(ohl[:], xlb[:], iot[0:VOC, :], None,
                            op0=AL.is_equal)
    psq = pst.tile([H, BPC], f32, tag="ps")
    nc.tensor.matmul(psq[:], tab2[0:VOC, 2 * H:3 * H], ohl[:],
                     start=True, stop=True)
    qng = smp.tile([H, BPC], f32, tag="qng")
    nc.scalar.copy(qng[:], psq[:])
    prd = pst.tile([H, BPC], f32, tag="ps")
    for e in range(BPC):
        nc.tensor.matmul(prd[:, e:e + 1], Mneg[:, e * H:(e + 1) * H],
                         qng[:, e:e + 1], start=True, stop=True)
    rd = smp.tile([H, BPC], f32, tag="rd")
    nc.scalar.activation(rd[:], prd[:], AF.Relu)
    pso = pst.tile([H, BPC], f32, tag="ps")
    nc.tensor.matmul(pso[:], wos[:], rd[:], start=True, stop=True)
    ot = smp.tile([H, BPC], f32, tag="ot")
    nc.vector.tensor_scalar(ot[:], pso[:], bos[:], None, op0=AL.add)
    nc.sync.dma_start(out_d, ot[:])


def _enable_jax_compile_cache():
    """Persistent XLA compile cache: repeat calls (and repeat processes)
    skip the expensive neuronx/walrus recompile of the embedded BIR."""
    import jax
    try:
        jax.config.update("jax_compilation_cache_dir", "/tmp/jaxcache")
        jax.config.update("jax_persistent_cache_min_compile_time_secs", 0.0)
        jax.config.update("jax_persistent_cache_min_entry_size_bytes", -1)
    except Exception:
        pass


_NC_CACHE = []


def build_nc():
    if _NC_CACHE:
        return _NC_CACHE[0]
    from concourse import bacc
    import concourse.tile as tile
    from contextlib import ExitStack
    nc = bacc.Bacc("TRN2", target_bir_lowering=False, debug=False,
                   num_devices=NCORE)
    with tile.TileContext(nc) as tc:
        with ExitStack() as ctx:
            _build(nc, tc, ctx)
    nc.compile()
    _NC_CACHE.append(nc)
    return nc


def make_in_maps(inputs):
    x = np.asarray(inputs["x"]).astype(np.int64)
    consts = {
        "embed": inputs["embed"], "w1": inputs["w1"],
        "b1": np.asarray(inputs["b1"]).reshape(2 * H, 1),
        "w2": inputs["w2"], "b2": np.asarray(inputs["b2"]).reshape(H, 1),
        "ln_g": np.asarray(inputs["ln_g"]).reshape(1, H),
        "ln_b": np.asarray(inputs["ln_b"]).reshape(1, H),
        "wk": inputs["wk"], "wv": inputs["wv"], "wq": inputs["wq"],
        "wo": inputs["wo"], "bo": np.asarray(inputs["bo"]).reshape(H, 1),
        "iota2": (np.arange(128) % 64).astype(np.float32).reshape(128, 1),
        "ident": np.eye(128, dtype=np.float32),
    }
    consts = {k: np.ascontiguousarray(np.asarray(v, dtype=np.float32))
              for k, v in consts.items()}
    in_maps = []
    for c in range(NCORE):
        m = dict(consts)
        m["xf"] = np.ascontiguousarray(
            x[c * BPC:(c + 1) * BPC].astype(np.float32))
        m["xl"] = np.ascontiguousarray(
            x[c * BPC:(c + 1) * BPC, LEFF - 1].astype(np.float32).reshape(1, BPC))
        in_maps.append(m)
    return in_maps


def kernel(**inputs):
    _enable_jax_compile_cache()
    from concourse.bass_utils import run_bass_kernel_spmd
    nc = build_nc()
    in_maps = make_in_maps(inputs)
    res = run_bass_kernel_spmd(nc, in_maps, list(range(NCORE)))
    outs = []
    for c in range(NCORE):
        o = np.asarray(res.results[c]["out"])   # [H, BPC]
        outs.append(o.T)
    return np.concatenate(outs, axis=0).astype(np.float32)



# revision 4
# speedup vs baseline: 12.9138x; 3.1430x over previous
"""DeltaOnlyModel Trainium2 kernel.

Pure data parallel over batch: 256 examples -> 8 cores x 32.
The per-token encoder collapses to 64-entry token tables (vocab=64, no
position mixing), computed on HOST (numpy f32) and shipped packed into a
single constant blob. The gated delta-rule scan runs in a hardware For_i
loop over chunks of C=128 steps: per-chunk per-example K/theta sequences
come from one-hot matmuls on the tensor engine (quadrant packed), the
sequential gate recurrence runs on the vector engine in [32 ex x 64 H]
layout with fused scalar_tensor_tensor ops, and cross-sub-chunk
corrections plus fast-weight (M) updates are per-example matmuls
accumulating in PSUM.  The For_i loop keeps the emitted program ~16x
smaller than full unrolling, which is what dominates the end-to-end
dispatch wall time (BIR serialization + compile) under axon.
"""

import os
import numpy as np

H = 64
VOC = 64
L = 2048
B = 256
NCORE = 8
BPC = B // NCORE          # 32 examples per core
C = 128                   # chunk length (steps)
SC = 8                    # sub-chunk length (solve window)
NSUB = C // SC
THR2 = 0.4 * 0.4
LN_EPS = 1e-5
NORM_EPS = 1e-12

# test hook: truncate the scan to fewer chunks (kernel then models a
# shorter sequence whose readout token is x[:, NCH*C-1])
NCH = int(os.environ.get("KERNEL_NCH", L // C))
LEFF = NCH * C

# constant-blob column layout
CB_IOT = 0                # p % 64
CB_IOTP = 1               # p
CB_TAB = 2                # [k_hat | -k_hat | th]  (2H+1 cols)
CB_VT = CB_TAB + 2 * H + 1
CB_WO = CB_VT + H
CB_BO = CB_WO + H
CB_IDR = CB_BO + 1        # row 0 carries 0..127 (ident row source)
CB_W = CB_IDR + 128


def _build(nc, tc, ctx):
    from concourse import mybir
    from concourse import bass
    f32 = mybir.dt.float32
    u8 = mybir.dt.uint8
    AL = mybir.AluOpType
    AF = mybir.ActivationFunctionType
    HB = BPC // 2   # examples per partition-half

    cb_d = nc.dram_tensor("cblob", [128, CB_W], f32, kind="ExternalInput").ap()
    xfu = nc.dram_tensor("xfu", [BPC, L], u8, kind="ExternalInput").ap()
    qng_d = nc.dram_tensor("qng", [H, BPC], f32, kind="ExternalInput").ap()
    out_d = nc.dram_tensor("out", [H, BPC], f32, kind="ExternalOutput").ap()

    aneg_d = nc.dram_tensor("aneg_d", [BPC, C, C], f32).ap()
    u_d = nc.dram_tensor("u_d", [C, BPC, H], f32).ap()
    r_d = nc.dram_tensor("r_d", [BPC, SC, H], f32).ap()

    cst = ctx.enter_context(tc.tile_pool(name="cst", bufs=1))
    big = ctx.enter_context(tc.tile_pool(name="big", bufs=1))
    dbl = ctx.enter_context(tc.tile_pool(name="dbl", bufs=1))
    sol = ctx.enter_context(tc.tile_pool(name="sol", bufs=3))
    smp = ctx.enter_context(tc.tile_pool(name="smp", bufs=3))
    stp = ctx.enter_context(tc.tile_pool(name="stp", bufs=2))
    rrp = ctx.enter_context(tc.tile_pool(name="rrp", bufs=2))
    pst = ctx.enter_context(tc.tile_pool(name="pst", bufs=4, space="PSUM"))
    psr = ctx.enter_context(tc.tile_pool(name="psr", bufs=1, space="PSUM"))

    # ============ constants ============
    cb = cst.tile([128, CB_W], f32)
    nc.sync.dma_start(cb[:], cb_d)
    iot = cb[:, CB_IOT:CB_IOT + 1]
    tab2 = cb[:, CB_TAB:CB_TAB + 2 * H + 1]
    vtab2 = cb[:, CB_VT:CB_VT + H]
    wos = cb[0:H, CB_WO:CB_WO + H]
    bos = cb[0:H, CB_BO:CB_BO + 1]
    idr = cst.tile([128, 128], f32)
    nc.sync.dma_start(idr[:], cb_d[0:1, CB_IDR:CB_IDR + 128]
                      .broadcast_to([128, 128]))
    idn = cst.tile([128, 128], f32)
    nc.vector.tensor_scalar(idn[:], idr[:], cb[:, CB_IOTP:CB_IOTP + 1], None,
                            op0=AL.is_equal)

    Mneg = cst.tile([H, BPC * H], f32)   # -M^T per example
    nc.vector.memzero(Mneg[:])

    qng = cst.tile([H, BPC], f32)
    nc.sync.dma_start(qng[:], qng_d)

    # ============ phase 1: chunks (hardware loop) ============
    with tc.For_i(0, LEFF, C) as iv:
        xb = big.tile([128, HB, C], u8, tag="xb")
        nc.sync.dma_start(xb[0:VOC, :, :],
                          xfu[0:HB, bass.ds(iv, C)].unsqueeze(0)
                          .broadcast_to([VOC, HB, C]))
        nc.sync.dma_start(xb[VOC:128, :, :],
                          xfu[HB:BPC, bass.ds(iv, C)].unsqueeze(0)
                          .broadcast_to([VOC, HB, C]))
        xbf = big.tile([128, HB, C], f32, tag="xbf")
        nc.vector.tensor_copy(xbf[:], xb[:])
        oh = big.tile([128, HB, C], f32, tag="oh")
        nc.vector.tensor_scalar(oh[:], xbf[:], iot, None, op0=AL.is_equal)

        kall = dbl.tile([C, BPC, 2 * H], f32, tag="kall")
        thcol_all = smp.tile([C, BPC], f32, tag="thcol")
        ktall = dbl.tile([H, BPC, C], f32, tag="ktall")
        ktnall = big.tile([H, BPC, C], f32, tag="ktnall")
        for g in range(BPC // 4):
            psa = pst.tile([C, 4, 2 * H], f32, tag="ps")
            psth = pst.tile([C, 4, 1], f32, tag="ps")
            psbT = pst.tile([H, 4, C], f32, tag="ps")
            psbTn = pst.tile([H, 4, C], f32, tag="ps")
            for j in range(4):
                e = g * 4 + j
                half = 0 if e < HB else VOC
                es = e if e < HB else e - HB
                ohs = oh[half:half + VOC, es, :]
                nc.tensor.matmul(psa[:, j, :], ohs,
                                 tab2[half:half + VOC, 0:2 * H],
                                 start=True, stop=True, tile_position=(half, 0))
                nc.tensor.matmul(psth[:, j, :], ohs,
                                 tab2[half:half + VOC, 2 * H:2 * H + 1],
                                 start=True, stop=True, tile_position=(half, 0))
                nc.tensor.matmul(psbT[:, j, :],
                                 tab2[half:half + VOC, 0:H], ohs,
                                 start=True, stop=True, tile_position=(half, 0))
                nc.tensor.matmul(psbTn[:, j, :],
                                 tab2[half:half + VOC, H:2 * H], ohs,
                                 start=True, stop=True, tile_position=(half, 0))
            nc.scalar.copy(kall[:, g * 4:(g + 1) * 4, :], psa[:])
            nc.scalar.copy(thcol_all[:, g * 4:(g + 1) * 4], psth[:, :, 0])
            nc.scalar.copy(ktall[:, g * 4:(g + 1) * 4, :], psbT[:])
            nc.scalar.copy(ktnall[:, g * 4:(g + 1) * 4, :], psbTn[:])

        thps = pst.tile([BPC, C], f32, tag="ps")
        nc.tensor.transpose(thps[:], thcol_all[:], idn[0:C, 0:C])
        thb = sol.tile([BPC, C], f32, tag="thb")
        nc.scalar.copy(thb[:], thps[:])

        for g in range(BPC // 4):
            pan = pst.tile([C, 4, C], f32, tag="ps")
            ansb = smp.tile([C, 4, C], f32, tag="ansb")
            for j in range(4):
                e = g * 4 + j
                nc.tensor.matmul(pan[:, j, :], ktall[:, e, :], ktnall[:, e, :],
                                 start=True, stop=True)
            nc.scalar.copy(ansb[:], pan[:])
            nc.sync.dma_start(aneg_d[g * 4:(g + 1) * 4].transpose([1, 0, 2]),
                              ansb[:])
        acols = sol.tile([BPC, NSUB * SC, SC], f32, tag="acols")
        for J in range(NSUB):
            nc.sync.dma_start(
                acols[:, J * SC:(J + 1) * SC, :],
                aneg_d[:, J * SC:(J + 1) * SC, J * SC:(J + 1) * SC])

        # r'' init: V gather, then -K M^T
        # PSUM group discipline: one start=True per bank (8 examples/bank)
        # per chunk; everything else accumulates via per-element has_written.
        rps = psr.tile([C, BPC, H], f32, tag="rps")
        for e in range(BPC):
            half = 0 if e < HB else VOC
            es = e if e < HB else e - HB
            ohs = oh[half:half + VOC, es, :]
            nc.tensor.matmul(rps[:, e, :], ohs, vtab2[half:half + VOC, :],
                             start=(e % 8 == 0), stop=False,
                             tile_position=(half, 0), skip_group_check=True)
        laststop = NSUB < 2
        for e in range(BPC):
            nc.tensor.matmul(rps[:, e, :], ktall[:, e, :],
                             Mneg[:, e * H:(e + 1) * H],
                             start=False, stop=(laststop and e % 8 == 7),
                             skip_group_check=True)

        uc = big.tile([C, BPC, H], f32, tag="uc")
        for J in range(NSUB):
            rrow = rrp.tile([C, BPC, H], f32, tag="rrow")
            nc.scalar.copy(rrow[:], rps[:])
            nc.sync.dma_start(r_d.transpose([1, 0, 2]),
                              rrow[J * SC:(J + 1) * SC, :, :])
            rb = sol.tile([BPC, SC, H], f32, tag="rb")
            nc.sync.dma_start(rb[:, 0:1, :], r_d[:, 0:1, :])
            nc.sync.dma_start(rb[:, 1:SC, :], r_d[:, 1:SC, :])

            for k in range(SC):
                t = J * SC + k
                dslot = rb[:, k, :]
                for s in range(k):
                    nc.vector.scalar_tensor_tensor(
                        dslot, rb[:, s, :], acols[:, J * SC + s, k:k + 1], dslot,
                        op0=AL.mult, op1=AL.add)
                nsc = smp.tile([BPC, H], f32, tag="nsc")
                ncol = smp.tile([BPC, 1], f32, tag="ncol")
                nc.vector.scalar_tensor_tensor(nsc[:], dslot, 1.0, dslot,
                                               op0=AL.bypass, op1=AL.mult,
                                               accum_out=ncol[:])
                nc.vector.scalar_tensor_tensor(
                    dslot, ncol[:].broadcast_to([BPC, H]), thb[:, t:t + 1],
                    dslot, op0=AL.is_gt, op1=AL.mult)
                if k == SC - 2:
                    nc.sync.dma_start(
                        u_d[J * SC:J * SC + SC - 1].transpose([1, 0, 2]),
                        rb[:, 0:SC - 1, :])
            nc.sync.dma_start(
                u_d[J * SC + SC - 1:(J + 1) * SC].transpose([1, 0, 2]),
                rb[:, SC - 1:SC, :])
            if J + 1 < NSUB:
                # 4-way packed strip corrections: example groups at
                # partition rows 0/32/64/96, future-rows-only stationary.
                J32 = ((J + 1) * SC // 64) * 64
                usub = stp.tile([128, BPC // 4, H], f32, tag="usub")
                strip = stp.tile([128, BPC // 4, C], f32, tag="strip")
                for q in range(4):
                    es, ee = q * 8, (q + 1) * 8
                    nc.sync.dma_start(usub[32 * q:32 * q + SC, :, :],
                                      u_d[J * SC:(J + 1) * SC, es:ee, :]
                                      .transpose([0, 1, 2]))
                    nc.sync.dma_start(
                        strip[32 * q:32 * q + SC, :, :],
                        aneg_d[es:ee, J * SC:(J + 1) * SC, :]
                        .transpose([1, 0, 2]))
                for e in range(BPC):
                    q, er = e // 8, e % 8
                    nc.tensor.matmul(rps[J32:C, e, :],
                                     strip[32 * q:32 * q + SC, er, J32:C],
                                     usub[32 * q:32 * q + SC, er, :],
                                     start=False,
                                     stop=(J == NSUB - 2 and e % 8 == 7),
                                     skip_group_check=True,
                                     tile_position=(32 * q, J32))

        nc.sync.dma_start(uc[:], u_d)
        dmp = psr.tile([H, BPC, H], f32, tag="rps")
        for e in range(BPC):
            nc.tensor.matmul(dmp[:, e, :], kall[:, e, H:2 * H], uc[:, e, :],
                             start=True, stop=True)
        nc.vector.tensor_add(Mneg[:], Mneg[:],
                             dmp[:].rearrange("j e h -> j (e h)"))

    # ============ phase 2: readout ============
    prd = pst.tile([H, BPC], f32, tag="ps")
    for e in range(BPC):
        nc.tensor.matmul(prd[:, e:e + 1], Mneg[:, e * H:(e + 1) * H],
                         qng[:, e:e + 1], start=True, stop=True)
    rd = smp.tile([H, BPC], f32, tag="rd")
    nc.scalar.activation(rd[:], prd[:], AF.Relu)
    pso = pst.tile([H, BPC], f32, tag="ps")
    nc.tensor.matmul(pso[:], wos, rd[:], start=True, stop=True)
    ot = smp.tile([H, BPC], f32, tag="ot")
    nc.vector.tensor_scalar(ot[:], pso[:], bos, None, op0=AL.add)
    nc.sync.dma_start(out_d, ot[:])


def _enable_jax_compile_cache():
    """Persistent XLA compile cache: repeat calls (and repeat processes)
    skip the expensive neuronx/walrus recompile of the embedded BIR."""
    import jax
    try:
        jax.config.update("jax_compilation_cache_dir", "/tmp/jaxcache")
        jax.config.update("jax_persistent_cache_min_compile_time_secs", 0.0)
        jax.config.update("jax_persistent_cache_min_entry_size_bytes", -1)
    except Exception:
        pass


_NC_CACHE = []


def build_nc():
    if _NC_CACHE:
        return _NC_CACHE[0]
    from concourse import bacc
    import concourse.tile as tile
    from contextlib import ExitStack
    nc = bacc.Bacc("TRN2", target_bir_lowering=False, debug=False,
                   num_devices=NCORE)
    with tile.TileContext(nc) as tc:
        with ExitStack() as ctx:
            _build(nc, tc, ctx)
    nc.compile()
    _NC_CACHE.append(nc)
    return nc


def _host_tables(inputs):
    """Token tables in numpy f32 — identical math to the reference encoder."""
    I = {k: np.asarray(v, np.float32) for k, v in inputs.items() if k != "x"}
    h0 = I["embed"]                                         # [V, H]
    ff = np.maximum(h0 @ I["w1"] + I["b1"], 0) @ I["w2"] + I["b2"]
    hh = h0 + ff
    mu = hh.mean(-1, keepdims=True, dtype=np.float32)
    var = hh.var(-1, keepdims=True, dtype=np.float32)
    h = (hh - mu) / np.sqrt(var + LN_EPS) * I["ln_g"] + I["ln_b"]
    ktab = h @ I["wk"]
    kn = np.maximum(np.linalg.norm(ktab, axis=-1, keepdims=True), NORM_EPS)
    ktab = (ktab / kn).astype(np.float32)
    vtab = (h @ I["wv"]).astype(np.float32)
    qtab = (h @ I["wq"]).astype(np.float32)
    th = (THR2 * (vtab * vtab).sum(-1)).astype(np.float32)  # [V]
    return ktab, vtab, qtab, th, I["wo"], I["bo"]


def make_in_maps(inputs):
    x = np.asarray(inputs["x"]).astype(np.int64)
    ktab, vtab, qtab, th, wo, bo = _host_tables(inputs)

    cb = np.zeros((128, CB_W), np.float32)
    cb[:, CB_IOT] = np.arange(128) % 64
    cb[:, CB_IOTP] = np.arange(128)
    tabs = np.concatenate([ktab, -ktab, th[:, None]], axis=1)  # [V, 2H+1]
    cb[0:VOC, CB_TAB:CB_TAB + 2 * H + 1] = tabs
    cb[VOC:128, CB_TAB:CB_TAB + 2 * H + 1] = tabs
    cb[0:VOC, CB_VT:CB_VT + H] = vtab
    cb[VOC:128, CB_VT:CB_VT + H] = vtab
    cb[0:H, CB_WO:CB_WO + H] = wo
    cb[0:H, CB_BO] = bo
    cb[0, CB_IDR:CB_IDR + 128] = np.arange(128)
    cb = np.ascontiguousarray(cb)

    in_maps = []
    for c in range(NCORE):
        xs = x[c * BPC:(c + 1) * BPC]
        qng = np.ascontiguousarray(-qtab[xs[:, LEFF - 1]].T)   # [H, BPC]
        m = {
            "cblob": cb,
            "xfu": np.ascontiguousarray(xs.astype(np.uint8)),
            "qng": qng,
        }
        in_maps.append(m)
    return in_maps


def kernel(**inputs):
    _enable_jax_compile_cache()
    from concourse.bass_utils import run_bass_kernel_spmd
    nc = build_nc()
    in_maps = make_in_maps(inputs)
    res = run_bass_kernel_spmd(nc, in_maps, list(range(NCORE)))
    outs = []
    for c in range(NCORE):
        o = np.asarray(res.results[c]["out"])   # [H, BPC]
        outs.append(o.T)
    return np.concatenate(outs, axis=0).astype(np.float32)
